# revision 5
# baseline (speedup 1.0000x reference)
"""ConvLSTM cell forward on 8 Trainium2 NeuronCores.

Problem: B=16, Cin=64, Chid=128, H=W=64, K=3 (SAME padding).
  ig = sigmoid(conv(x,Wxi) + bxi + conv(h,Whi) + Wci*c)
  fg = sigmoid(conv(x,Wxf) + bxf + conv(h,Whf) + Wcf*c)
  c_new = fg*c + ig*tanh(conv(x,Wxc) + bxc + conv(h,Whc))
  og = sigmoid(conv(x,Wxo) + bxo + conv(h,Who) + Wco*c)
  h_new = og*c_new
  returns (og, h_new, c_new)

Strategy:
  - Data-parallel over batch: 2 images per core, weights replicated.
  - Conv as matmul over channel dim: inputs stored channel-on-partition with
    a zero-padded (H+2)x(W+2) spatial layout flattened with row stride 66.
    A 3x3 tap (dy,dx) is then a constant flat offset, so each tap is one
    contiguous matmul rhs slice accumulating into PSUM.
  - h convs: Chid=128 channels -> 9 taps of K=128 matmuls per gate.
  - x convs: Cin=64 gives only 64 contraction lanes per tap, so taps are
    packed in pairs onto 128 partitions. Two stacked input arrangements:
      xp: partitions 0-63 x_pad, 64-127 x_pad shifted one padded ROW (+66)
          -> one matmul covers taps (0,dx)+(1,dx): 3 matmuls.
      xq: partitions 0-63 x_pad, 64-127 x_pad shifted one COLUMN (+1)
          -> one matmul covers taps (2,0)+(2,1); tap (2,2) rides a final
          matmul whose upper 64 weight rows are zero. 2 matmuls.
    5 K=128 x-matmuls per gate (vs 4.5 ideal, 6 before).
  - DMA issue is split across both HWDGE queues: weights on the Activation
    (scalar) queue, image data + outputs on the SP (sync) queue, so the
    ~600ns/DMA descriptor-generation serial cost doesn't delay the first
    matmul. Gate-3 (candidate) and gate-0 weights are split into small
    tiles so tap-0 matmuls start as soon as ~100KB has landed.
  - Gates are processed candidate-first, output-gate-last so the post-last-
    matmul tail is just pre-add -> sigmoid -> h_new mul -> DMA.
  - Output computed in the padded-stride layout, rows chunked so N<=512 per
    PSUM bank; the 2 garbage columns per row are skipped by strided views
    in the elementwise stage.
  - Matmuls in fp16 (inputs/weights pre-rounded on host), accumulate fp32.
"""

import os
import numpy as np

B, CIN, CHID, H, W, K = 16, 64, 128, 64, 64, 3
N_CORES = 8
PER = B // N_CORES          # images per core
WPAD = W + 2                # padded row stride
FLAT = (H + 2) * WPAD + 4   # padded flat length (+4 tail pad for tap overread)
# output chunks: (start_row, n_rows); N = n_rows*66 must be even, <=512
CHUNKS = [(r, 7) for r in range(0, 56, 7)] + [(56, 4), (60, 4)]
HW = H * W

# gate processing order: candidate first (tanh can start early), output gate
# last (shortest tail after the final matmul). Index meaning: 0=i 1=f 2=o 3=c
GORDER = [3, 0, 1, 2]
# all weight tiles split small: whole-gate 295KB DMAs cost ~1.4us of
# descriptor-generation each and arrived after the tensor engine needed them
H_SPLIT = {3: 3, 0: 3, 1: 3, 2: 3}   # weight tile pieces per gate (9 taps)
X_SPLIT = {3: 2, 0: 2, 1: 2, 2: 2}   # weight tile pieces per gate (5 blocks)

_PROG = None
LAST_RESULTS = None


def _pad_flat(a):
    """[N, C, H, W] fp32 -> [N, C, FLAT] zero-padded 66-stride layout."""
    n, c = a.shape[0], a.shape[1]
    out = np.zeros((n, c, FLAT), dtype=np.float32)
    p = out[:, :, : (H + 2) * WPAD].reshape(n, c, H + 2, WPAD)
    p[:, :, 1 : H + 1, 1 : W + 1] = a
    return out


def _build_program():
    import concourse.bacc as bacc
    import concourse.tile as tile
    import concourse.mybir as mybir
    from contextlib import ExitStack

    f32 = mybir.dt.float32
    f16 = mybir.dt.float16

    nc = bacc.Bacc("TRN2", target_bir_lowering=False, debug=False,
                   num_devices=N_CORES)

    xp_d = nc.dram_tensor("xp", [PER, 2 * CIN, FLAT], f16, kind="ExternalInput").ap()
    hp_d = nc.dram_tensor("hp", [PER, CHID, FLAT], f16, kind="ExternalInput").ap()
    c_d = nc.dram_tensor("c", [PER, CHID, HW], f32, kind="ExternalInput").ap()
    # x weights: 5 K=128 tap-blocks per gate (3 row-pairs, 1 col-pair, 1 single)
    wx_d = nc.dram_tensor("wx", [4, CHID, 5 * CHID], f16, kind="ExternalInput").ap()
    wh_d = nc.dram_tensor("wh", [4, CHID, 9 * CHID], f16, kind="ExternalInput").ap()
    bias_d = nc.dram_tensor("bias", [CHID, 4], f32, kind="ExternalInput").ap()
    peep_d = nc.dram_tensor("peep", [3, CHID, HW], f32, kind="ExternalInput").ap()
    og_d = nc.dram_tensor("og", [PER, CHID, HW], f32, kind="ExternalOutput").ap()
    hn_d = nc.dram_tensor("hn", [PER, CHID, HW], f32, kind="ExternalOutput").ap()
    cn_d = nc.dram_tensor("cn", [PER, CHID, HW], f32, kind="ExternalOutput").ap()

    SIG = mybir.ActivationFunctionType.Sigmoid
    TANH = mybir.ActivationFunctionType.Tanh

    with tile.TileContext(nc) as tc, ExitStack() as ctx:
        const = ctx.enter_context(tc.tile_pool(name="const", bufs=1))
        imgs = ctx.enter_context(tc.tile_pool(name="imgs", bufs=2))
        work = ctx.enter_context(tc.tile_pool(name="work", bufs=2))
        outs = ctx.enter_context(tc.tile_pool(name="outs", bufs=2))
        psum = ctx.enter_context(tc.tile_pool(name="psum", bufs=8, space="PSUM"))

        # ---- weight tiles, issued on the Activation HWDGE queue in the
        # exact order the tensor engine will consume them ----
        wh_t = {}   # gate -> list of (tile, tap0, ntaps)
        wx_t = {}   # gate -> list of (tile, blk0, nblks)
        for g in GORDER:
            wh_t[g] = [(const.tile([CHID, 3 * CHID], f16, name=f"wh{g}_{p}"),
                        3 * p, 3) for p in range(3)]
            wx_t[g] = [(const.tile([CHID, 3 * CHID], f16, name=f"wx{g}_0"),
                        0, 3),
                       (const.tile([CHID, 2 * CHID], f16, name=f"wx{g}_1"),
                        3, 2)]
            for t, tap0, n in wh_t[g]:
                nc.scalar.dma_start(t[:], wh_d[g][:, tap0 * CHID:(tap0 + n) * CHID])
            for t, b0, n in wx_t[g]:
                nc.scalar.dma_start(t[:], wx_d[g][:, b0 * CHID:(b0 + n) * CHID])

        bias_t = const.tile([CHID, 4], f32)
        nc.scalar.dma_start(bias_t[:], bias_d)
        peep_t = [const.tile([CHID, HW], f32, tag=f"peep{j}", name=f"peep{j}")
                  for j in range(3)]
        # interleaved halves so peep0 (needed first) completes earliest
        for s in (0, HW // 2):
            for j in range(3):
                nc.scalar.dma_start(peep_t[j][:, s:s + HW // 2],
                                    peep_d[j][:, s:s + HW // 2])

        def wslice(pieces, idx, width):
            """weight AP for logical block `idx` of width CHID cols."""
            for t, i0, n in pieces:
                if i0 <= idx < i0 + n:
                    return t[:, (idx - i0) * width:(idx - i0 + 1) * width]
            raise AssertionError

        for b in range(PER):
            # image loads on the SP HWDGE queue, 3 pieces each, ordered so
            # the earliest-consumed bytes land first
            xp = imgs.tile([2 * CIN, FLAT], f16, tag="xp", name=f"xp{b}")
            hp = imgs.tile([CHID, FLAT], f16, tag="hp", name=f"hp{b}")
            xq = imgs.tile([2 * CIN, FLAT], f16, tag="xq", name=f"xq{b}")
            # small first piece so chunk-0 matmuls start as early as possible
            bounds = [0, 600, 1854, 3107, FLAT]
            for s, e in zip(bounds, bounds[1:]):
                nc.sync.dma_start(hp[:, s:e], hp_d[b][:, s:e])
                nc.sync.dma_start(xp[:, s:e], xp_d[b][:, s:e])
                nc.sync.dma_start(xq[0:CIN, s:e], xp_d[b][0:CIN, s:e])
                e2 = min(e, FLAT - 1)
                nc.sync.dma_start(xq[CIN:2 * CIN, s:e2],
                                  xp_d[b][0:CIN, s + 1:e2 + 1])

            for kc, (row0, nrows) in enumerate(CHUNKS):
                o0 = row0 * WPAD
                cn_mm = nrows * WPAD
                cc = nrows * W
                ps = {}
                for g in GORDER:
                    p = psum.tile([CHID, cn_mm], f32, tag="ps",
                                  padded_shape=[CHID, 512],
                                  name=f"ps{b}_{kc}_{g}")
                    ps[g] = p
                    for tap in range(9):
                        dy, dx = divmod(tap, 3)
                        off = o0 + dy * WPAD + dx
                        nc.tensor.matmul(
                            p[:], wslice(wh_t[g], tap, CHID),
                            hp[:, off:off + cn_mm],
                            start=(tap == 0), stop=False)
                    # x blocks: 0-2 pair (0,dx)+(1,dx) via xp; 3 pairs
                    # (2,0)+(2,1) via xq; 4 is (2,2) with zero upper half
                    for j in range(5):
                        if j < 3:
                            off = o0 + j
                            src = xp
                        elif j == 3:
                            off = o0 + 2 * WPAD
                            src = xq
                        else:
                            off = o0 + 2 * WPAD + 2
                            src = xq
                        nc.tensor.matmul(
                            p[:], wslice(wx_t[g], j, CHID),
                            src[:, off:off + cn_mm],
                            start=False, stop=(j == 4))

                def pv(p):  # valid-region view of a psum chunk [128, nrows, W]
                    return p[:].rearrange("p (r c) -> p r c", c=WPAD)[:, :, 0:W]

                def v3(t):  # [128, cc] compact -> [128, nrows, W]
                    return t.rearrange("p (r c) -> p r c", c=W)

                c0 = row0 * W
                ctc = outs.tile([CHID, cc], f32, tag="ct", bufs=3,
                                padded_shape=[CHID, 448],
                                name=f"ct{b}_{kc}")
                nc.sync.dma_start(ctc[:], c_d[b][:, c0:c0 + cc])
                csl = ctc[:]
                # peephole products can run as soon as c arrives
                pe = []
                for gi in range(3):
                    t = work.tile([CHID, cc], f32, tag=f"pe{gi}",
                                  padded_shape=[CHID, 448],
                                  name=f"pe{b}_{kc}_{gi}")
                    nc.vector.tensor_mul(t[:], peep_t[gi][:, c0:c0 + cc], csl)
                    pe.append(t)
                # candidate gate: tanh straight off PSUM
                gc = work.tile([CHID, cc], f32, tag="gc",
                               padded_shape=[CHID, 448])
                nc.scalar.activation(v3(gc[:]), pv(ps[3]), TANH,
                                     bias=bias_t[:, 3:4])

                acts = []
                for gi in range(3):  # i, f, o with peephole + sigmoid
                    pre = work.tile([CHID, cc], f32, tag=f"pre{gi}",
                                    padded_shape=[CHID, 448],
                                    name=f"pre{b}_{kc}_{gi}")
                    nc.vector.tensor_add(v3(pre[:]), pv(ps[gi]), v3(pe[gi][:]))
                    act = work.tile([CHID, cc], f32, tag=f"act{gi}",
                                    padded_shape=[CHID, 448],
                                    name=f"act{b}_{kc}_{gi}")
                    nc.scalar.activation(act[:], pre[:], SIG,
                                         bias=bias_t[:, gi:gi + 1])
                    acts.append(act)
                ig, fg, og = acts

                t2 = work.tile([CHID, cc], f32, tag="t2",
                               padded_shape=[CHID, 448])
                nc.vector.tensor_mul(t2[:], ig[:], gc[:])
                t1 = work.tile([CHID, cc], f32, tag="t1",
                               padded_shape=[CHID, 448])
                nc.vector.tensor_mul(t1[:], fg[:], csl)
                cn = outs.tile([CHID, cc], f32, tag="cn",
                               padded_shape=[CHID, 448])
                nc.vector.tensor_add(cn[:], t1[:], t2[:])

                sl = slice(c0, c0 + cc)
                nc.sync.dma_start(og_d[b][:, sl], og[:])
                nc.sync.dma_start(cn_d[b][:, sl], cn[:])
                hn = outs.tile([CHID, cc], f32, tag="hn",
                               padded_shape=[CHID, 448])
                nc.vector.tensor_mul(hn[:], og[:], cn[:])
                nc.sync.dma_start(hn_d[b][:, sl], hn[:])

    nc.compile()
    return nc


def kernel(x, h, c, Wxi, bxi, Whi, Wci, Wxf, bxf, Whf, Wcf,
           Wxo, bxo, Who, Wco, Wxc, bxc, Whc):
    global _PROG, LAST_RESULTS
    from concourse.bass_utils import run_bass_kernel_spmd

    x = np.asarray(x, dtype=np.float32)
    h = np.asarray(h, dtype=np.float32)
    c = np.asarray(c, dtype=np.float32)

    # x: padded layout duplicated on the channel axis, second copy shifted
    # one padded row so a K=128 matmul covers (dy=0, dy=1) tap pairs
    xpad = _pad_flat(x)
    xp = np.zeros((B, 2 * CIN, FLAT), dtype=np.float32)
    xp[:, :CIN] = xpad
    xp[:, CIN:, : FLAT - WPAD] = xpad[:, :, WPAD:]
    xp = xp.astype(np.float16)
    hp = _pad_flat(h).astype(np.float16)
    cf = np.ascontiguousarray(c.reshape(B, CHID, HW))

    def wx_prep(w):
        # [Co=128, Ci=64, 3, 3] -> [128, 5*128]: blocks 0-2 stack the
        # (dy=0,dx) tap over (dy=1,dx); block 3 stacks (2,0) over (2,1);
        # block 4 holds (2,2) over zeros
        w = np.asarray(w, dtype=np.float32)
        out = np.zeros((CHID, 5 * CHID), dtype=np.float32)
        for dx in range(3):
            out[:CIN, dx * CHID:(dx + 1) * CHID] = w[:, :, 0, dx].T
            out[CIN:, dx * CHID:(dx + 1) * CHID] = w[:, :, 1, dx].T
        out[:CIN, 3 * CHID:4 * CHID] = w[:, :, 2, 0].T
        out[CIN:, 3 * CHID:4 * CHID] = w[:, :, 2, 1].T
        out[:CIN, 4 * CHID:5 * CHID] = w[:, :, 2, 2].T
        return out.astype(np.float16)

    def wh_prep(w):
        w = np.asarray(w, dtype=np.float32)
        return np.ascontiguousarray(
            w.transpose(1, 2, 3, 0).reshape(CHID, 9 * CHID)).astype(np.float16)

    wx = np.stack([wx_prep(Wxi), wx_prep(Wxf), wx_prep(Wxo), wx_prep(Wxc)])
    wh = np.stack([wh_prep(Whi), wh_prep(Whf), wh_prep(Who), wh_prep(Whc)])
    bias = np.ascontiguousarray(np.stack(
        [np.asarray(v, dtype=np.float32) for v in (bxi, bxf, bxo, bxc)], axis=1))
    peep = np.stack([np.asarray(v, dtype=np.float32).reshape(CHID, HW)
                     for v in (Wci, Wcf, Wco)])

    if _PROG is None:
        _PROG = _build_program()

    in_maps = []
    for i in range(N_CORES):
        sl = slice(i * PER, (i + 1) * PER)
        in_maps.append({
            "xp": np.ascontiguousarray(xp[sl]),
            "hp": np.ascontiguousarray(hp[sl]),
            "c": np.ascontiguousarray(cf[sl]),
            "wx": wx, "wh": wh, "bias": bias, "peep": peep,
        })

    res = run_bass_kernel_spmd(nc=_PROG, in_maps=in_maps,
                               core_ids=list(range(N_CORES)),
                               trace=bool(os.environ.get("KERNEL_TRACE")))
    LAST_RESULTS = res

    og = np.empty((B, CHID, HW), dtype=np.float32)
    hn = np.empty((B, CHID, HW), dtype=np.float32)
    cn = np.empty((B, CHID, HW), dtype=np.float32)
    for i in range(N_CORES):
        sl = slice(i * PER, (i + 1) * PER)
        og[sl] = res.results[i]["og"]
        hn[sl] = res.results[i]["hn"]
        cn[sl] = res.results[i]["cn"]

    shape = (B, CHID, H, W)
    return (og.reshape(shape), hn.reshape(shape), cn.reshape(shape))


# revision 7
# speedup vs baseline: 1.3612x; 1.3612x over previous
"""ConvLSTM cell forward on 8 Trainium2 NeuronCores.

Problem: B=16, Cin=64, Chid=128, H=W=64, K=3 (SAME padding).
  ig = sigmoid(conv(x,Wxi) + bxi + conv(h,Whi) + Wci*c)
  fg = sigmoid(conv(x,Wxf) + bxf + conv(h,Whf) + Wcf*c)
  c_new = fg*c + ig*tanh(conv(x,Wxc) + bxc + conv(h,Whc))
  og = sigmoid(conv(x,Wxo) + bxo + conv(h,Who) + Wco*c)
  h_new = og*c_new
  returns (og, h_new, c_new)

Strategy:
  - Data-parallel over batch: 2 images per core, weights replicated.
  - Conv as matmul over channel dim: inputs stored channel-on-partition with
    a zero-padded (H+2)x(W+2) spatial layout flattened with row stride 66.
    A 3x3 tap (dy,dx) is then a constant flat offset, so each tap is one
    contiguous matmul rhs slice accumulating into PSUM.
  - h convs: Chid=128 channels -> 9 taps of K=128 matmuls per gate.
  - x convs: Cin=64 gives only 64 contraction lanes per tap, so taps are
    packed in pairs onto 128 partitions. Two stacked input arrangements:
      xp: partitions 0-63 x_pad, 64-127 x_pad shifted one padded ROW (+66)
          -> one matmul covers taps (0,dx)+(1,dx): 3 matmuls.
      xq: partitions 0-63 x_pad, 64-127 x_pad shifted one COLUMN (+1)
          -> one matmul covers taps (2,0)+(2,1); tap (2,2) rides a final
          matmul whose upper 64 weight rows are zero. 2 matmuls.
    5 K=128 x-matmuls per gate (vs 4.5 ideal, 6 before).
  - DMA issue is split across both HWDGE queues: weights on the Activation
    (scalar) queue, image data + outputs on the SP (sync) queue, so the
    ~600ns/DMA descriptor-generation serial cost doesn't delay the first
    matmul. Gate-3 (candidate) and gate-0 weights are split into small
    tiles so tap-0 matmuls start as soon as ~100KB has landed.
  - Gates are processed candidate-first, output-gate-last so the post-last-
    matmul tail is just pre-add -> sigmoid -> h_new mul -> DMA.
  - Output computed in the padded-stride layout, rows chunked so N<=512 per
    PSUM bank; the 2 garbage columns per row are skipped by strided views
    in the elementwise stage.
  - Matmuls in fp16 (inputs/weights pre-rounded on host), accumulate fp32.
"""

import os
import numpy as np

B, CIN, CHID, H, W, K = 16, 64, 128, 64, 64, 3
N_CORES = 8
PER = B // N_CORES          # images per core
WPAD = W + 2                # padded row stride
FLAT = (H + 2) * WPAD + 4   # padded flat length (+4 tail pad for tap overread)
# output chunks: (start_row, n_rows); N = n_rows*66 must be even, <=512
CHUNKS = [(r, 7) for r in range(0, 56, 7)] + [(56, 4), (60, 4)]
HW = H * W

# gate processing order: candidate first (tanh can start early), output gate
# last (shortest tail after the final matmul). Index meaning: 0=i 1=f 2=o 3=c
GORDER = [3, 0, 1, 2]
# all weight tiles split small: whole-gate 295KB DMAs cost ~1.4us of
# descriptor-generation each and arrived after the tensor engine needed them
H_SPLIT = {3: 3, 0: 3, 1: 3, 2: 3}   # weight tile pieces per gate (9 taps)
X_SPLIT = {3: 2, 0: 2, 1: 2, 2: 2}   # weight tile pieces per gate (5 blocks)

_PROG = None
LAST_RESULTS = None


def _pad_flat(a):
    """[N, C, H, W] fp32 -> [N, C, FLAT] zero-padded 66-stride layout."""
    n, c = a.shape[0], a.shape[1]
    out = np.zeros((n, c, FLAT), dtype=np.float32)
    p = out[:, :, : (H + 2) * WPAD].reshape(n, c, H + 2, WPAD)
    p[:, :, 1 : H + 1, 1 : W + 1] = a
    return out


def _build_program():
    import concourse.bacc as bacc
    import concourse.tile as tile
    import concourse.mybir as mybir
    from contextlib import ExitStack

    f32 = mybir.dt.float32
    f16 = mybir.dt.float16

    nc = bacc.Bacc("TRN2", target_bir_lowering=False, debug=False,
                   num_devices=N_CORES)

    xp_d = nc.dram_tensor("xp", [PER, 2 * CIN, FLAT], f16, kind="ExternalInput").ap()
    hp_d = nc.dram_tensor("hp", [PER, CHID, FLAT], f16, kind="ExternalInput").ap()
    c_d = nc.dram_tensor("c", [PER, CHID, HW], f32, kind="ExternalInput").ap()
    # x weights: 5 K=128 tap-blocks per gate (3 row-pairs, 1 col-pair, 1 single)
    wx_d = nc.dram_tensor("wx", [4, CHID, 5 * CHID], f16, kind="ExternalInput").ap()
    wh_d = nc.dram_tensor("wh", [4, CHID, 9 * CHID], f16, kind="ExternalInput").ap()
    bias_d = nc.dram_tensor("bias", [CHID, 4], f32, kind="ExternalInput").ap()
    peep_d = nc.dram_tensor("peep", [3, CHID, HW], f32, kind="ExternalInput").ap()
    og_d = nc.dram_tensor("og", [PER, CHID, HW], f32, kind="ExternalOutput").ap()
    hn_d = nc.dram_tensor("hn", [PER, CHID, HW], f32, kind="ExternalOutput").ap()
    cn_d = nc.dram_tensor("cn", [PER, CHID, HW], f32, kind="ExternalOutput").ap()

    SIG = mybir.ActivationFunctionType.Sigmoid
    TANH = mybir.ActivationFunctionType.Tanh

    with tile.TileContext(nc) as tc, ExitStack() as ctx:
        const = ctx.enter_context(tc.tile_pool(name="const", bufs=1))
        imgs = ctx.enter_context(tc.tile_pool(name="imgs", bufs=2))
        work = ctx.enter_context(tc.tile_pool(name="work", bufs=2))
        outs = ctx.enter_context(tc.tile_pool(name="outs", bufs=2))
        psum = ctx.enter_context(tc.tile_pool(name="psum", bufs=8, space="PSUM"))

        # ---- weight tiles, issued on the Activation HWDGE queue in the
        # exact order the tensor engine will consume them ----
        wh_t = {}   # gate -> list of (tile, tap0, ntaps)
        wx_t = {}   # gate -> list of (tile, blk0, nblks)
        for g in GORDER:
            wh_t[g] = [(const.tile([CHID, 3 * CHID], f16, name=f"wh{g}_{p}"),
                        3 * p, 3) for p in range(3)]
            wx_t[g] = [(const.tile([CHID, 3 * CHID], f16, name=f"wx{g}_0"),
                        0, 3),
                       (const.tile([CHID, 2 * CHID], f16, name=f"wx{g}_1"),
                        3, 2)]
            for t, tap0, n in wh_t[g]:
                nc.scalar.dma_start(t[:], wh_d[g][:, tap0 * CHID:(tap0 + n) * CHID])
            for t, b0, n in wx_t[g]:
                nc.scalar.dma_start(t[:], wx_d[g][:, b0 * CHID:(b0 + n) * CHID])

        bias_t = const.tile([CHID, 4], f32)
        nc.scalar.dma_start(bias_t[:], bias_d)
        peep_t = [const.tile([CHID, HW], f32, tag=f"peep{j}", name=f"peep{j}")
                  for j in range(3)]
        # interleaved halves so peep0 (needed first) completes earliest
        for s in (0, HW // 2):
            for j in range(3):
                nc.scalar.dma_start(peep_t[j][:, s:s + HW // 2],
                                    peep_d[j][:, s:s + HW // 2])

        def wslice(pieces, idx, width):
            """weight AP for logical block `idx` of width CHID cols."""
            for t, i0, n in pieces:
                if i0 <= idx < i0 + n:
                    return t[:, (idx - i0) * width:(idx - i0 + 1) * width]
            raise AssertionError

        # image tiles for both images; image-1's loads are emitted midway
        # through image-0's schedule so they never queue behind image-0's
        # final output DMAs on the in-order sync queue
        xpt = {b: imgs.tile([2 * CIN, FLAT], f16, tag="xp", name=f"xp{b}")
               for b in range(PER)}
        hpt = {b: imgs.tile([CHID, FLAT], f16, tag="hp", name=f"hp{b}")
               for b in range(PER)}
        xqt = {b: imgs.tile([2 * CIN, FLAT], f16, tag="xq", name=f"xq{b}")
               for b in range(PER)}

        def emit_pieces(b):
            bounds = [0, 600, 1854, 3107, FLAT]
            for s, e in zip(bounds, bounds[1:]):
                nc.sync.dma_start(hpt[b][:, s:e], hp_d[b][:, s:e])
                nc.sync.dma_start(xpt[b][:, s:e], xp_d[b][:, s:e])
                nc.sync.dma_start(xqt[b][0:CIN, s:e], xp_d[b][0:CIN, s:e])
                e2 = min(e, FLAT - 1)
                nc.sync.dma_start(xqt[b][CIN:2 * CIN, s:e2],
                                  xp_d[b][0:CIN, s + 1:e2 + 1])

        emit_pieces(0)

        for b in range(PER):
            xp, hp, xq = xpt[b], hpt[b], xqt[b]

            # gate-major over half-images: one gate's weights cover ~13us of
            # work, so the scalar-queue weight DMAs are never on the critical
            # path (chunk-major stalled ~5us waiting for gate 1/2 weights)
            for hf, hchunks in enumerate((CHUNKS[:5], CHUNKS[5:])):
                cts, gcs, t2s, cns = {}, {}, {}, {}

                def pv(p, nrows):  # psum valid view [128, nrows, W]
                    return p[:].rearrange("p (r c) -> p r c",
                                          c=WPAD)[:, :, 0:W]

                def v3(t, nrows):  # [128, cc] compact -> [128, nrows, W]
                    return t.rearrange("p (r c) -> p r c", c=W)

                for g in GORDER:  # 3 (cand), 0 (i), 1 (f), 2 (o)
                    for kc, (row0, nrows) in enumerate(hchunks):
                        o0 = row0 * WPAD
                        cn_mm = nrows * WPAD
                        cc = nrows * W
                        c0 = row0 * W
                        ps = psum.tile([CHID, cn_mm], f32, tag="ps",
                                       padded_shape=[CHID, 512],
                                       name=f"ps{b}_{hf}_{g}_{kc}")
                        for tap in range(9):
                            dy, dx = divmod(tap, 3)
                            off = o0 + dy * WPAD + dx
                            nc.tensor.matmul(
                                ps[:], wslice(wh_t[g], tap, CHID),
                                hp[:, off:off + cn_mm],
                                start=(tap == 0), stop=False)
                        for j in range(5):
                            if j < 3:
                                off = o0 + j
                                src = xp
                            elif j == 3:
                                off = o0 + 2 * WPAD
                                src = xq
                            else:
                                off = o0 + 2 * WPAD + 2
                                src = xq
                            nc.tensor.matmul(
                                ps[:], wslice(wx_t[g], j, CHID),
                                src[:, off:off + cn_mm],
                                start=False, stop=(j == 4))

                        if b == 0 and hf == 1 and g == 0 and kc == 0 \
                                and PER > 1:
                            emit_pieces(1)
                        if g == 3:
                            # prefetch c one gate-pass (~13us) ahead
                            ctc = outs.tile([CHID, cc], f32, tag=f"ct{kc}",
                                            bufs=1, padded_shape=[CHID, 448],
                                            name=f"ct{b}_{hf}_{kc}")
                            nc.sync.dma_start(ctc[:], c_d[b][:, c0:c0 + cc])
                            cts[kc] = ctc
                            gc = work.tile([CHID, cc], f32, tag=f"gc{kc}",
                                           bufs=1, padded_shape=[CHID, 448],
                                           name=f"gc{b}_{hf}_{kc}")
                            nc.scalar.activation(v3(gc[:], nrows),
                                                 pv(ps, nrows), TANH,
                                                 bias=bias_t[:, 3:4])
                            gcs[kc] = gc
                            continue

                        pe = work.tile([CHID, cc], f32, tag="pe",
                                       padded_shape=[CHID, 448],
                                       name=f"pe{b}_{hf}_{g}_{kc}")
                        nc.vector.tensor_mul(pe[:],
                                             peep_t[g][:, c0:c0 + cc],
                                             cts[kc][:])
                        pre = work.tile([CHID, cc], f32, tag="pre",
                                        padded_shape=[CHID, 448],
                                        name=f"pr{b}_{hf}_{g}_{kc}")
                        nc.vector.tensor_add(v3(pre[:], nrows), pv(ps, nrows),
                                             v3(pe[:], nrows))
                        act = work.tile([CHID, cc], f32, tag=f"act{g}",
                                        padded_shape=[CHID, 448],
                                        name=f"ac{b}_{hf}_{g}_{kc}")
                        nc.scalar.activation(act[:], pre[:], SIG,
                                             bias=bias_t[:, g:g + 1])

                        if g == 0:      # i-gate: t2 = ig * tanh(cand)
                            t2 = work.tile([CHID, cc], f32, tag=f"t2{kc}",
                                           bufs=1, padded_shape=[CHID, 448],
                                           name=f"t2{b}_{hf}_{kc}")
                            nc.vector.tensor_mul(t2[:], act[:], gcs[kc][:])
                            t2s[kc] = t2
                        elif g == 1:    # f-gate: c_new done, store it now
                            t1 = work.tile([CHID, cc], f32, tag="t1",
                                           padded_shape=[CHID, 448],
                                           name=f"t1{b}_{hf}_{kc}")
                            nc.vector.tensor_mul(t1[:], act[:], cts[kc][:])
                            cn = outs.tile([CHID, cc], f32, tag=f"cn{kc}",
                                           bufs=1, padded_shape=[CHID, 448],
                                           name=f"cn{b}_{hf}_{kc}")
                            nc.vector.tensor_add(cn[:], t1[:], t2s[kc][:])
                            nc.sync.dma_start(cn_d[b][:, c0:c0 + cc], cn[:])
                            cns[kc] = cn
                        else:           # o-gate: og, h_new, store
                            nc.sync.dma_start(og_d[b][:, c0:c0 + cc], act[:])
                            hn = work.tile([CHID, cc], f32, tag="hn",
                                           padded_shape=[CHID, 448],
                                           name=f"hn{b}_{hf}_{kc}")
                            nc.vector.tensor_mul(hn[:], act[:], cns[kc][:])
                            # hn rides the scalar HWDGE queue: sync is busy
                            # with og and the scalar queue is idle by now
                            nc.scalar.dma_start(hn_d[b][:, c0:c0 + cc], hn[:])

    nc.compile()
    return nc


def kernel(x, h, c, Wxi, bxi, Whi, Wci, Wxf, bxf, Whf, Wcf,
           Wxo, bxo, Who, Wco, Wxc, bxc, Whc):
    global _PROG, LAST_RESULTS
    from concourse.bass_utils import run_bass_kernel_spmd

    x = np.asarray(x, dtype=np.float32)
    h = np.asarray(h, dtype=np.float32)
    c = np.asarray(c, dtype=np.float32)

    # x: padded layout duplicated on the channel axis, second copy shifted
    # one padded row so a K=128 matmul covers (dy=0, dy=1) tap pairs
    xpad = _pad_flat(x)
    xp = np.zeros((B, 2 * CIN, FLAT), dtype=np.float32)
    xp[:, :CIN] = xpad
    xp[:, CIN:, : FLAT - WPAD] = xpad[:, :, WPAD:]
    xp = xp.astype(np.float16)
    hp = _pad_flat(h).astype(np.float16)
    cf = np.ascontiguousarray(c.reshape(B, CHID, HW))

    def wx_prep(w):
        # [Co=128, Ci=64, 3, 3] -> [128, 5*128]: blocks 0-2 stack the
        # (dy=0,dx) tap over (dy=1,dx); block 3 stacks (2,0) over (2,1);
        # block 4 holds (2,2) over zeros
        w = np.asarray(w, dtype=np.float32)
        out = np.zeros((CHID, 5 * CHID), dtype=np.float32)
        for dx in range(3):
            out[:CIN, dx * CHID:(dx + 1) * CHID] = w[:, :, 0, dx].T
            out[CIN:, dx * CHID:(dx + 1) * CHID] = w[:, :, 1, dx].T
        out[:CIN, 3 * CHID:4 * CHID] = w[:, :, 2, 0].T
        out[CIN:, 3 * CHID:4 * CHID] = w[:, :, 2, 1].T
        out[:CIN, 4 * CHID:5 * CHID] = w[:, :, 2, 2].T
        return out.astype(np.float16)

    def wh_prep(w):
        w = np.asarray(w, dtype=np.float32)
        return np.ascontiguousarray(
            w.transpose(1, 2, 3, 0).reshape(CHID, 9 * CHID)).astype(np.float16)

    wx = np.stack([wx_prep(Wxi), wx_prep(Wxf), wx_prep(Wxo), wx_prep(Wxc)])
    wh = np.stack([wh_prep(Whi), wh_prep(Whf), wh_prep(Who), wh_prep(Whc)])
    bias = np.ascontiguousarray(np.stack(
        [np.asarray(v, dtype=np.float32) for v in (bxi, bxf, bxo, bxc)], axis=1))
    peep = np.stack([np.asarray(v, dtype=np.float32).reshape(CHID, HW)
                     for v in (Wci, Wcf, Wco)])

    if _PROG is None:
        _PROG = _build_program()

    in_maps = []
    for i in range(N_CORES):
        sl = slice(i * PER, (i + 1) * PER)
        in_maps.append({
            "xp": np.ascontiguousarray(xp[sl]),
            "hp": np.ascontiguousarray(hp[sl]),
            "c": np.ascontiguousarray(cf[sl]),
            "wx": wx, "wh": wh, "bias": bias, "peep": peep,
        })

    res = run_bass_kernel_spmd(nc=_PROG, in_maps=in_maps,
                               core_ids=list(range(N_CORES)),
                               trace=bool(os.environ.get("KERNEL_TRACE")))
    LAST_RESULTS = res

    og = np.empty((B, CHID, HW), dtype=np.float32)
    hn = np.empty((B, CHID, HW), dtype=np.float32)
    cn = np.empty((B, CHID, HW), dtype=np.float32)
    for i in range(N_CORES):
        sl = slice(i * PER, (i + 1) * PER)
        og[sl] = res.results[i]["og"]
        hn[sl] = res.results[i]["hn"]
        cn[sl] = res.results[i]["cn"]

    shape = (B, CHID, H, W)
    return (og.reshape(shape), hn.reshape(shape), cn.reshape(shape))


# revision 9
# speedup vs baseline: 1.3866x; 1.0186x over previous
"""ConvLSTM cell forward on 8 Trainium2 NeuronCores.

Problem: B=16, Cin=64, Chid=128, H=W=64, K=3 (SAME padding).
  ig = sigmoid(conv(x,Wxi) + bxi + conv(h,Whi) + Wci*c)
  fg = sigmoid(conv(x,Wxf) + bxf + conv(h,Whf) + Wcf*c)
  c_new = fg*c + ig*tanh(conv(x,Wxc) + bxc + conv(h,Whc))
  og = sigmoid(conv(x,Wxo) + bxo + conv(h,Who) + Wco*c)
  h_new = og*c_new
  returns (og, h_new, c_new)

Strategy:
  - Data-parallel over batch: 2 images per core, weights replicated.
  - Conv as matmul over channel dim: inputs stored channel-on-partition with
    a zero-padded (H+2)x(W+2) spatial layout flattened with row stride 66.
    A 3x3 tap (dy,dx) is then a constant flat offset, so each tap is one
    contiguous matmul rhs slice accumulating into PSUM.
  - h convs: Chid=128 channels -> 9 taps of K=128 matmuls per gate.
  - x convs: Cin=64 gives only 64 contraction lanes per tap, so taps are
    packed in pairs onto 128 partitions. Two stacked input arrangements:
      xp: partitions 0-63 x_pad, 64-127 x_pad shifted one padded ROW (+66)
          -> one matmul covers taps (0,dx)+(1,dx): 3 matmuls.
      xq: partitions 0-63 x_pad, 64-127 x_pad shifted one COLUMN (+1)
          -> one matmul covers taps (2,0)+(2,1); tap (2,2) rides a final
          matmul whose upper 64 weight rows are zero. 2 matmuls.
    5 K=128 x-matmuls per gate (vs 4.5 ideal, 6 before).
  - DMA issue is split across both HWDGE queues: weights on the Activation
    (scalar) queue, image data + outputs on the SP (sync) queue, so the
    ~600ns/DMA descriptor-generation serial cost doesn't delay the first
    matmul. Gate-3 (candidate) and gate-0 weights are split into small
    tiles so tap-0 matmuls start as soon as ~100KB has landed.
  - Gates are processed candidate-first, output-gate-last so the post-last-
    matmul tail is just pre-add -> sigmoid -> h_new mul -> DMA.
  - Output computed in the padded-stride layout, rows chunked so N<=512 per
    PSUM bank; the 2 garbage columns per row are skipped by strided views
    in the elementwise stage.
  - Matmuls in fp16 (inputs/weights pre-rounded on host), accumulate fp32.
"""

import os
import numpy as np

B, CIN, CHID, H, W, K = 16, 64, 128, 64, 64, 3
N_CORES = 8
PER = B // N_CORES          # images per core
WPAD = W + 2                # padded row stride
FLAT = (H + 2) * WPAD + 4   # padded flat length (+4 tail pad for tap overread)
# output chunks: (start_row, n_rows); N = n_rows*66 must be even, <=512
CHUNKS = [(r, 7) for r in range(0, 56, 7)] + [(56, 4), (60, 4)]
HW = H * W

# gate processing order: candidate first (tanh can start early), output gate
# last (shortest tail after the final matmul). Index meaning: 0=i 1=f 2=o 3=c
GORDER = [3, 0, 1, 2]
# all weight tiles split small: whole-gate 295KB DMAs cost ~1.4us of
# descriptor-generation each and arrived after the tensor engine needed them
H_SPLIT = {3: 3, 0: 3, 1: 3, 2: 3}   # weight tile pieces per gate (9 taps)
X_SPLIT = {3: 2, 0: 2, 1: 2, 2: 2}   # weight tile pieces per gate (5 blocks)

_PROG = None
LAST_RESULTS = None


def _pad_flat(a):
    """[N, C, H, W] fp32 -> [N, C, FLAT] zero-padded 66-stride layout."""
    n, c = a.shape[0], a.shape[1]
    out = np.zeros((n, c, FLAT), dtype=np.float32)
    p = out[:, :, : (H + 2) * WPAD].reshape(n, c, H + 2, WPAD)
    p[:, :, 1 : H + 1, 1 : W + 1] = a
    return out


def _build_program():
    import concourse.bacc as bacc
    import concourse.tile as tile
    import concourse.mybir as mybir
    from contextlib import ExitStack

    f32 = mybir.dt.float32
    f16 = mybir.dt.float16

    nc = bacc.Bacc("TRN2", target_bir_lowering=False, debug=False,
                   num_devices=N_CORES)

    xp_d = nc.dram_tensor("xp", [PER, 2 * CIN, FLAT], f16, kind="ExternalInput").ap()
    hp_d = nc.dram_tensor("hp", [PER, CHID, FLAT], f16, kind="ExternalInput").ap()
    c_d = nc.dram_tensor("c", [PER, CHID, HW], f32, kind="ExternalInput").ap()
    # x weights: 5 K=128 tap-blocks per gate (3 row-pairs, 1 col-pair, 1 single)
    wx_d = nc.dram_tensor("wx", [4, CHID, 5 * CHID], f16, kind="ExternalInput").ap()
    wh_d = nc.dram_tensor("wh", [4, CHID, 9 * CHID], f16, kind="ExternalInput").ap()
    bias_d = nc.dram_tensor("bias", [CHID, 4], f32, kind="ExternalInput").ap()
    peep_d = nc.dram_tensor("peep", [3, CHID, HW], f32, kind="ExternalInput").ap()
    og_d = nc.dram_tensor("og", [PER, CHID, HW], f32, kind="ExternalOutput").ap()
    hn_d = nc.dram_tensor("hn", [PER, CHID, HW], f32, kind="ExternalOutput").ap()
    cn_d = nc.dram_tensor("cn", [PER, CHID, HW], f32, kind="ExternalOutput").ap()

    SIG = mybir.ActivationFunctionType.Sigmoid
    TANH = mybir.ActivationFunctionType.Tanh

    with tile.TileContext(nc) as tc, ExitStack() as ctx:
        const = ctx.enter_context(tc.tile_pool(name="const", bufs=1))
        imgs = ctx.enter_context(tc.tile_pool(name="imgs", bufs=2))
        work = ctx.enter_context(tc.tile_pool(name="work", bufs=2))
        outs = ctx.enter_context(tc.tile_pool(name="outs", bufs=2))
        psum = ctx.enter_context(tc.tile_pool(name="psum", bufs=8, space="PSUM"))

        # ---- weight tiles, issued on the Activation HWDGE queue in the
        # exact order the tensor engine will consume them ----
        wh_t = {}   # gate -> list of (tile, tap0, ntaps)
        wx_t = {}   # gate -> list of (tile, blk0, nblks)
        def issue_w(g, eng):
            for t, tap0, n in wh_t[g]:
                eng.dma_start(t[:], wh_d[g][:, tap0 * CHID:(tap0 + n) * CHID])
            for t, b0, n in wx_t[g]:
                eng.dma_start(t[:], wx_d[g][:, b0 * CHID:(b0 + n) * CHID])

        for g in GORDER:
            wh_t[g] = [(const.tile([CHID, 3 * CHID], f16, name=f"wh{g}_{p}"),
                        3 * p, 3) for p in range(3)]
            wx_t[g] = [(const.tile([CHID, 3 * CHID], f16, name=f"wx{g}_0"),
                        0, 3),
                       (const.tile([CHID, 2 * CHID], f16, name=f"wx{g}_1"),
                        3, 2)]
            # gates 3 and 0 ride the scalar queue; gates 1 and 2 ride the
            # sync queue interleaved with the image pieces (below) — one
            # queue alone takes ~17us to issue all 28 pieces and stalled
            # the tensor engine ~4us waiting for gate-1/2 weights
            if g in (3, 0):
                issue_w(g, nc.scalar)

        bias_t = const.tile([CHID, 4], f32)
        nc.scalar.dma_start(bias_t[:], bias_d)
        peep_t = [const.tile([CHID, HW], f32, tag=f"peep{j}", name=f"peep{j}")
                  for j in range(3)]
        # interleaved halves so peep0 (needed first) completes earliest
        for s in (0, HW // 2):
            for j in range(3):
                nc.scalar.dma_start(peep_t[j][:, s:s + HW // 2],
                                    peep_d[j][:, s:s + HW // 2])

        def wslice(pieces, idx, width):
            """weight AP for logical block `idx` of width CHID cols."""
            for t, i0, n in pieces:
                if i0 <= idx < i0 + n:
                    return t[:, (idx - i0) * width:(idx - i0 + 1) * width]
            raise AssertionError

        for b in range(PER):
            # image loads on the SP HWDGE queue, 3 pieces each, ordered so
            # the earliest-consumed bytes land first
            xp = imgs.tile([2 * CIN, FLAT], f16, tag="xp", name=f"xp{b}")
            hp = imgs.tile([CHID, FLAT], f16, tag="hp", name=f"hp{b}")
            xq = imgs.tile([2 * CIN, FLAT], f16, tag="xq", name=f"xq{b}")
            # small first piece so chunk-0 matmuls start as early as possible
            bounds = [0, 600, 1854, 3107, FLAT]

            def img_piece(i, parts="hxq"):
                s, e = bounds[i], bounds[i + 1]
                if "h" in parts:
                    nc.sync.dma_start(hp[:, s:e], hp_d[b][:, s:e])
                if "x" in parts:
                    nc.sync.dma_start(xp[:, s:e], xp_d[b][:, s:e])
                if "q" in parts:
                    nc.sync.dma_start(xq[0:CIN, s:e], xp_d[b][0:CIN, s:e])
                    e2 = min(e, FLAT - 1)
                    nc.sync.dma_start(xq[CIN:2 * CIN, s:e2],
                                      xp_d[b][0:CIN, s + 1:e2 + 1])

            if b == 0:
                # urgent bytes first, then gate-1/2 weights slotted where
                # the sync queue would otherwise move low-urgency pieces
                img_piece(0)
                img_piece(1, "hx")
                issue_w(1, nc.sync)
                img_piece(1, "q")
                issue_w(2, nc.sync)
                img_piece(2)
                img_piece(3)
            else:
                for i in range(4):
                    img_piece(i)

            for kc, (row0, nrows) in enumerate(CHUNKS):
                o0 = row0 * WPAD
                cn_mm = nrows * WPAD
                cc = nrows * W
                ps = {}
                for g in GORDER:
                    p = psum.tile([CHID, cn_mm], f32, tag="ps",
                                  padded_shape=[CHID, 512],
                                  name=f"ps{b}_{kc}_{g}")
                    ps[g] = p
                    for tap in range(9):
                        dy, dx = divmod(tap, 3)
                        off = o0 + dy * WPAD + dx
                        nc.tensor.matmul(
                            p[:], wslice(wh_t[g], tap, CHID),
                            hp[:, off:off + cn_mm],
                            start=(tap == 0), stop=False)
                    # x blocks: 0-2 pair (0,dx)+(1,dx) via xp; 3 pairs
                    # (2,0)+(2,1) via xq; 4 is (2,2) with zero upper half
                    for j in range(5):
                        if j < 3:
                            off = o0 + j
                            src = xp
                        elif j == 3:
                            off = o0 + 2 * WPAD
                            src = xq
                        else:
                            off = o0 + 2 * WPAD + 2
                            src = xq
                        nc.tensor.matmul(
                            p[:], wslice(wx_t[g], j, CHID),
                            src[:, off:off + cn_mm],
                            start=False, stop=(j == 4))

                def pv(p):  # valid-region view of a psum chunk [128, nrows, W]
                    return p[:].rearrange("p (r c) -> p r c", c=WPAD)[:, :, 0:W]

                def v3(t):  # [128, cc] compact -> [128, nrows, W]
                    return t.rearrange("p (r c) -> p r c", c=W)

                c0 = row0 * W
                ctc = outs.tile([CHID, cc], f32, tag="ct", bufs=3,
                                padded_shape=[CHID, 448],
                                name=f"ct{b}_{kc}")
                nc.sync.dma_start(ctc[:], c_d[b][:, c0:c0 + cc])
                csl = ctc[:]
                # peephole products can run as soon as c arrives
                pe = []
                for gi in range(3):
                    t = work.tile([CHID, cc], f32, tag=f"pe{gi}",
                                  padded_shape=[CHID, 448],
                                  name=f"pe{b}_{kc}_{gi}")
                    nc.vector.tensor_mul(t[:], peep_t[gi][:, c0:c0 + cc], csl)
                    pe.append(t)
                # candidate gate: tanh straight off PSUM
                gc = work.tile([CHID, cc], f32, tag="gc",
                               padded_shape=[CHID, 448])
                nc.scalar.activation(v3(gc[:]), pv(ps[3]), TANH,
                                     bias=bias_t[:, 3:4])

                acts = []
                for gi in range(3):  # i, f, o with peephole + sigmoid
                    pre = work.tile([CHID, cc], f32, tag=f"pre{gi}",
                                    padded_shape=[CHID, 448],
                                    name=f"pre{b}_{kc}_{gi}")
                    nc.vector.tensor_add(v3(pre[:]), pv(ps[gi]), v3(pe[gi][:]))
                    act = work.tile([CHID, cc], f32, tag=f"act{gi}",
                                    padded_shape=[CHID, 448],
                                    name=f"act{b}_{kc}_{gi}")
                    nc.scalar.activation(act[:], pre[:], SIG,
                                         bias=bias_t[:, gi:gi + 1])
                    acts.append(act)
                ig, fg, og = acts

                t2 = work.tile([CHID, cc], f32, tag="t2",
                               padded_shape=[CHID, 448])
                nc.vector.tensor_mul(t2[:], ig[:], gc[:])
                t1 = work.tile([CHID, cc], f32, tag="t1",
                               padded_shape=[CHID, 448])
                nc.vector.tensor_mul(t1[:], fg[:], csl)
                cn = outs.tile([CHID, cc], f32, tag="cn",
                               padded_shape=[CHID, 448])
                nc.vector.tensor_add(cn[:], t1[:], t2[:])

                sl = slice(c0, c0 + cc)
                nc.sync.dma_start(og_d[b][:, sl], og[:])
                nc.sync.dma_start(cn_d[b][:, sl], cn[:])
                hn = outs.tile([CHID, cc], f32, tag="hn",
                               padded_shape=[CHID, 448])
                nc.vector.tensor_mul(hn[:], og[:], cn[:])
                nc.sync.dma_start(hn_d[b][:, sl], hn[:])

    nc.compile()
    return nc


def kernel(x, h, c, Wxi, bxi, Whi, Wci, Wxf, bxf, Whf, Wcf,
           Wxo, bxo, Who, Wco, Wxc, bxc, Whc):
    global _PROG, LAST_RESULTS
    from concourse.bass_utils import run_bass_kernel_spmd

    x = np.asarray(x, dtype=np.float32)
    h = np.asarray(h, dtype=np.float32)
    c = np.asarray(c, dtype=np.float32)

    # x: padded layout duplicated on the channel axis, second copy shifted
    # one padded row so a K=128 matmul covers (dy=0, dy=1) tap pairs
    xpad = _pad_flat(x)
    xp = np.zeros((B, 2 * CIN, FLAT), dtype=np.float32)
    xp[:, :CIN] = xpad
    xp[:, CIN:, : FLAT - WPAD] = xpad[:, :, WPAD:]
    xp = xp.astype(np.float16)
    hp = _pad_flat(h).astype(np.float16)
    cf = np.ascontiguousarray(c.reshape(B, CHID, HW))

    def wx_prep(w):
        # [Co=128, Ci=64, 3, 3] -> [128, 5*128]: blocks 0-2 stack the
        # (dy=0,dx) tap over (dy=1,dx); block 3 stacks (2,0) over (2,1);
        # block 4 holds (2,2) over zeros
        w = np.asarray(w, dtype=np.float32)
        out = np.zeros((CHID, 5 * CHID), dtype=np.float32)
        for dx in range(3):
            out[:CIN, dx * CHID:(dx + 1) * CHID] = w[:, :, 0, dx].T
            out[CIN:, dx * CHID:(dx + 1) * CHID] = w[:, :, 1, dx].T
        out[:CIN, 3 * CHID:4 * CHID] = w[:, :, 2, 0].T
        out[CIN:, 3 * CHID:4 * CHID] = w[:, :, 2, 1].T
        out[:CIN, 4 * CHID:5 * CHID] = w[:, :, 2, 2].T
        return out.astype(np.float16)

    def wh_prep(w):
        w = np.asarray(w, dtype=np.float32)
        return np.ascontiguousarray(
            w.transpose(1, 2, 3, 0).reshape(CHID, 9 * CHID)).astype(np.float16)

    wx = np.stack([wx_prep(Wxi), wx_prep(Wxf), wx_prep(Wxo), wx_prep(Wxc)])
    wh = np.stack([wh_prep(Whi), wh_prep(Whf), wh_prep(Who), wh_prep(Whc)])
    bias = np.ascontiguousarray(np.stack(
        [np.asarray(v, dtype=np.float32) for v in (bxi, bxf, bxo, bxc)], axis=1))
    peep = np.stack([np.asarray(v, dtype=np.float32).reshape(CHID, HW)
                     for v in (Wci, Wcf, Wco)])

    if _PROG is None:
        _PROG = _build_program()

    in_maps = []
    for i in range(N_CORES):
        sl = slice(i * PER, (i + 1) * PER)
        in_maps.append({
            "xp": np.ascontiguousarray(xp[sl]),
            "hp": np.ascontiguousarray(hp[sl]),
            "c": np.ascontiguousarray(cf[sl]),
            "wx": wx, "wh": wh, "bias": bias, "peep": peep,
        })

    res = run_bass_kernel_spmd(nc=_PROG, in_maps=in_maps,
                               core_ids=list(range(N_CORES)),
                               trace=bool(os.environ.get("KERNEL_TRACE")))
    LAST_RESULTS = res

    og = np.empty((B, CHID, HW), dtype=np.float32)
    hn = np.empty((B, CHID, HW), dtype=np.float32)
    cn = np.empty((B, CHID, HW), dtype=np.float32)
    for i in range(N_CORES):
        sl = slice(i * PER, (i + 1) * PER)
        og[sl] = res.results[i]["og"]
        hn[sl] = res.results[i]["hn"]
        cn[sl] = res.results[i]["cn"]

    shape = (B, CHID, H, W)
    return (og.reshape(shape), hn.reshape(shape), cn.reshape(shape))
